# revision 32
# baseline (speedup 1.0000x reference)
"""Trainium2 Bass kernel for nn_CenterTOpEXnewMultiC (vq_codebook, K=2).

Strategy: shard the N = w*h pixel dimension across 8 cores (8192 px each);
each core streams its [8, 128, 8192] FeatureT slice once (memory-bound).

Per batch image b on a core (pixels n local to the slice):
  - s[k, n] = <centerInit[k], X[:, n]>  via 64 PE matmuls with the DATA as
    the stationary operand: lhsT = X[:, j::64] ([c=128, m=128], free stride
    64) and rhs = centerInit.T [128, 2] moving.  Output pairs land in PSUM
    [m=128, 2j+k] <-> pixel n = j + 64*m, i.e. partition m holds the
    pair-interleaved flat range [128m, 128m+128) -- exactly the raw-reshape
    layout dist2centerTout/labels_onehotout need.  One contiguous DMA out.
  - dists = (s-1)*(-0.5) on DVE (bit-equal to 0.5*(1-s)).
  - label = (d1 < d0), onehot0 = (d0 <= d1) on DVE -- compares on the
    rounded dists, matching jnp.argmin tie semantics exactly.
  - labels cast to int32 on DVE.
Only batch 7 feeds centersIterout, so only there: repack labels to a
[1, 8192] row, broadcast across partitions via a K=1 PE matmul with ones,
then DVE multiply+reduce per 512-chunk for S1[c] = sum_n X[c,n]*l[n],
plus plain reduces for T[c] = sum_n X[c,n] and the label count.  Host
combines the per-core partials into the [2, 128] centersIterout.

Batch 7 is processed FIRST so its extra stats work overlaps the stream of
batches 0-6; the LAST batch's three output DMAs are spread across
gpsimd/scalar/sync so their descriptor emissions overlap in the tail.
TimelineSim: 104.1 us/core vs 102.2 us memory roofline.
"""

import numpy as np
from contextlib import ExitStack

import concourse.bass as bass
import concourse.bacc as bacc
import concourse.tile as tile
from concourse import mybir
from concourse.bass_utils import run_bass_kernel_spmd

N_CORES = 8
BB = 8
C = 128
W = H = 256
N_FULL = W * H            # 65536 pixels per batch image
NS = N_FULL // N_CORES    # 8192 pixels per core
M = 128                   # stationary columns per matmul
CHUNK = 512               # chunk for batch-7 stats ops
DMA_SPLIT = 1             # input DMAs per batch (every matmul reads the whole
                          # x tile via a stride-64 AP, so more DMAs = more sem
                          # waits per matmul; walrus caps those)


def build_nc(bb=BB, c=C, ns=NS, enable_stats=True, enable_compares=True,
             enable_matmul=True, stats_mode="full", in_engine="sync",
             out_engine="gpsimd", nq=1, dma_split=4, xbufs=3,
             tail_engines=("scalar", "gpsimd", "sync")):
    j = ns // M
    nchunk = ns // CHUNK
    f32 = mybir.dt.float32
    AX = mybir.AxisListType.X
    ALU = mybir.AluOpType

    # Bacc (not plain Bass): its finalize() runs move_matmul_waits_to_ldweights
    # + generate_event_semaphores, which legalize instructions down to the
    # 1-sync-wait HW limit (walrus otherwise dies with "Too many sync wait
    # commands" on the first matmul, which waits on two DMA sems).
    nc = bacc.Bacc("TRN2", num_swdge_queues=nq)
    feat = nc.declare_dram_parameter("feat", [bb, c, ns], f32, isOutput=False)
    cent = nc.declare_dram_parameter("cent", [c, 2], f32, isOutput=False)
    dists_o = nc.declare_dram_parameter("dists_o", [bb, 2 * ns], f32, isOutput=True)
    oneh_o = nc.declare_dram_parameter("oneh_o", [bb, 2 * ns], f32, isOutput=True)
    lab_o = nc.declare_dram_parameter("lab_o", [bb, ns], mybir.dt.int32, isOutput=True)
    stats_o = nc.declare_dram_parameter("stats_o", [c, 4], f32, isOutput=True)

    eng_in = getattr(nc, in_engine)
    eng_out = nc.gpsimd if out_engine == "mixed" else getattr(nc, out_engine)
    with tile.TileContext(nc) as tc, ExitStack() as ctx:
        xpool = ctx.enter_context(tc.tile_pool(name="xpool", bufs=xbufs))
        opool = ctx.enter_context(tc.tile_pool(name="opool", bufs=3))
        scr = ctx.enter_context(tc.tile_pool(name="scr", bufs=2))
        consts = ctx.enter_context(tc.tile_pool(name="consts", bufs=1))
        pairsp = ctx.enter_context(tc.tile_pool(name="pairsp", bufs=3, space="PSUM"))
        bcastp = ctx.enter_context(tc.tile_pool(name="bcastp", bufs=4, space="PSUM"))

        cent_sb = consts.tile([c, 2], f32)
        nc.gpsimd.dma_start(out=cent_sb[:], in_=cent[:])
        ones_sb = consts.tile([1, M], f32)
        nc.vector.memset(ones_sb[:], 1.0)
        s1_part = consts.tile([c, nchunk], f32)
        t_part = consts.tile([c, nchunk], f32)
        stats_sb = consts.tile([c, 4], f32)
        nc.vector.memset(stats_sb[:], 0.0)

        order = [bb - 1] + list(range(bb - 1))
        for b in order:
            x_sb = xpool.tile([c, ns], f32)
            dstep = ns // dma_split
            for q in range(dma_split):
                eng_in.dma_start(
                    out=x_sb[:, q * dstep:(q + 1) * dstep],
                    in_=feat[b, :, q * dstep:(q + 1) * dstep],
                )
            # pixel n = jj + 64*m; lhsT slice jj: pixels {jj + 64*m}
            x_r = x_sb[:].rearrange("c (m jj) -> c jj m", jj=j)
            dpair = opool.tile([M, 2 * j], f32)
            opair = opool.tile([M, 2 * j], f32)
            if enable_matmul:
                pairs_ps = pairsp.tile([M, 2 * j], f32)
                for jj in range(j):
                    nc.tensor.matmul(
                        pairs_ps[:, 2 * jj:2 * jj + 2],
                        x_r[:, jj, :],
                        cent_sb[:],
                        start=True,
                        stop=True,
                    )
                # dists = (s - 1)*(-0.5) on DVE: bit-equal to 0.5*(1-s)
                # (RN(s-1) = -RN(1-s); *0.5 exact), and keeps the whole
                # dists->compares chain on one engine (no ACT hop in the tail).
                nc.vector.tensor_scalar(
                    out=dpair[:],
                    in0=pairs_ps[:],
                    scalar1=1.0,
                    scalar2=-0.5,
                    op0=ALU.subtract,
                    op1=ALU.mult,
                )
            else:
                nc.scalar.activation(
                    out=dpair[:],
                    in_=x_sb[:, 0:2 * j],
                    func=mybir.ActivationFunctionType.Copy,
                    bias=0.5,
                    scale=-0.5,
                )
            dv = dpair[:].rearrange("m (jj k) -> m k jj", k=2)
            l_f32 = opool.tile([M, j], f32)
            lab_sb = opool.tile([M, j], mybir.dt.int32)
            if enable_compares:
                nc.vector.tensor_tensor(
                    out=l_f32[:], in0=dv[:, 1, :], in1=dv[:, 0, :], op=ALU.is_lt
                )
                ov = opair[:].rearrange("m (jj k) -> m k jj", k=2)
                nc.vector.tensor_tensor(
                    out=ov[:, 0, :], in0=dv[:, 0, :], in1=dv[:, 1, :], op=ALU.is_le
                )
                nc.vector.tensor_copy(out=ov[:, 1, :], in_=l_f32[:])
                nc.vector.tensor_copy(out=lab_sb[:], in_=l_f32[:])
            else:
                nc.vector.memset(l_f32[:], 0.0)
                nc.vector.memset(opair[:], 0.0)
                nc.vector.memset(lab_sb[:], 0)

            d_out = dists_o[b].rearrange("(p f) -> p f", p=M)
            o_out = oneh_o[b].rearrange("(p f) -> p f", p=M)
            l_out = lab_o[b].rearrange("(p f) -> p f", p=M)
            last = b == order[-1]
            # Final batch: spread the three output DMAs across three engines
            # so their descriptor emissions overlap (the input stream is done
            # by then, so borrowing sync/scalar is free).
            te = [getattr(nc, e) for e in tail_engines]
            (te[0] if last else eng_out).dma_start(out=d_out, in_=dpair[:])
            (te[1] if last else eng_out).dma_start(out=o_out, in_=opair[:])
            (te[2] if last else eng_out).dma_start(out=l_out, in_=lab_sb[:])

            if b == bb - 1 and not enable_stats:
                eng_out.dma_start(out=stats_o[:], in_=stats_sb[:])
            if b == bb - 1 and enable_stats:
                # l_f32[m, jj] <-> pixel 64*m + jj: flat order matches pixels.
                label_row = consts.tile([1, ns], f32)
                if stats_mode == "no_gather":
                    nc.vector.memset(label_row[:], 0.0)
                else:
                    nc.gpsimd.dma_start(
                        out=label_row[:].rearrange("p (m jj) -> p m jj", jj=j),
                        in_=l_f32[:],
                    )
                for q in range(nchunk):
                    lb_ps = bcastp.tile([M, CHUNK], f32)
                    if stats_mode == "no_bcast":
                        nc.vector.memset(lb_ps[:], 0.0)
                    else:
                        nc.tensor.matmul(
                            lb_ps[:],
                            ones_sb[:],
                            label_row[:, q * CHUNK:(q + 1) * CHUNK],
                            start=True,
                            stop=True,
                        )
                    # NOTE: InstTensorTensorReduce passes CoreSim but crashes
                    # the NEFF at runtime on this stack -- use mult + reduce.
                    prod = scr.tile([c, CHUNK], f32)
                    nc.vector.tensor_tensor(
                        out=prod[:],
                        in0=x_sb[:, q * CHUNK:(q + 1) * CHUNK],
                        in1=lb_ps[:],
                        op=ALU.mult,
                    )
                    nc.vector.reduce_sum(
                        out=s1_part[:, q:q + 1],
                        in_=prod[:],
                        axis=AX,
                    )
                    nc.vector.reduce_sum(
                        out=t_part[:, q:q + 1],
                        in_=x_sb[:, q * CHUNK:(q + 1) * CHUNK],
                        axis=AX,
                    )
                nc.vector.reduce_sum(out=stats_sb[:, 0:1], in_=s1_part[:], axis=AX)
                nc.vector.reduce_sum(out=stats_sb[:, 1:2], in_=t_part[:], axis=AX)
                nc.vector.reduce_sum(out=stats_sb[:, 2:3], in_=l_f32[:], axis=AX)
                eng_out.dma_start(out=stats_o[:], in_=stats_sb[:])
    nc.finalize()
    return nc


_nc_cache = None


def _get_nc():
    global _nc_cache
    if _nc_cache is None:
        _nc_cache = build_nc()
    return _nc_cache


def _run(in_maps, trace=False):
    return run_bass_kernel_spmd(_get_nc(), in_maps, list(range(N_CORES)), trace=trace)


def make_in_maps(FeatureT, centerInit):
    FT = np.ascontiguousarray(np.asarray(FeatureT, dtype=np.float32)).reshape(
        BB, C, N_FULL
    )
    centT = np.ascontiguousarray(np.asarray(centerInit, dtype=np.float32).T)
    in_maps = []
    for m in range(N_CORES):
        sl = np.ascontiguousarray(FT[:, :, m * NS:(m + 1) * NS])
        in_maps.append({"feat": sl, "cent": centT})
    return in_maps


def assemble(results, centerInit):
    centerInit = np.asarray(centerInit, dtype=np.float32)
    dists = np.concatenate(
        [r["dists_o"].reshape(BB, 1, 2 * NS) for r in results], axis=1
    ).reshape(BB, 2, W, H)
    oneh = np.concatenate(
        [r["oneh_o"].reshape(BB, 1, 2 * NS) for r in results], axis=1
    ).reshape(BB, 2, W, H)
    labs = (
        np.concatenate([r["lab_o"].reshape(BB, 1, NS) for r in results], axis=1)
        .reshape(BB, 1, W, H)
        .astype(np.int32)
    )
    S1 = np.zeros(C, np.float64)
    T = np.zeros(C, np.float64)
    cnt1 = 0.0
    for r in results:
        st = r["stats_o"].astype(np.float64)
        S1 += st[:, 0]
        T += st[:, 1]
        cnt1 += st[:, 2].sum()
    S0 = T - S1
    centersIter7 = np.stack(
        [S0 / (N_FULL - cnt1 + 1.0), S1 / (cnt1 + 1.0)]
    ).astype(np.float32)
    centersIterout = (centerInit + 0.001 * (centersIter7 - centerInit)).astype(
        np.float32
    )
    return centersIterout, labs, oneh, dists, labs


def kernel(FeatureT, centerInit, num1=None, num2=None):
    in_maps = make_in_maps(FeatureT, centerInit)
    res = _run(in_maps).results
    return assemble(res, centerInit)
